# revision 11
# baseline (speedup 1.0000x reference)
"""Trainium2 Bass kernel for CrossAttention (RoPE, 16 heads, D=1024).

Sharding: data-parallel over (batch, query-half): core c handles batch c//2,
query rows [1024*(c%2), 1024*(c%2+1)).  Each core computes full k/v for its
batch; no cross-core communication; the host gather is a concatenation.

v2 design (all SBUF-resident, fp8 DoubleRow attention):
  - q/k features live in a "DR layout": head h = (g, s) (quad g=h//4, slot
    s=h%4); chip feature block (g, t) holds, on partition rows s*32+j, the
    RoPE half t of head h (t=0: even orig idx, t=1: odd).  RoPE's half-swap
    is then a free-dim offset, not a partition shuffle.
  - scores: fp8e4 DoubleRow matmuls, K=32 x 2 k-tiles (=head dim 64), per
    (head, kv-block): out [128 kv, 512 q] at 0.5 cyc/col.
  - E = exp(S/8) written as fp8e4 pairs; PV uses fp8 DoubleRow over kv-block
    pairs with a ones column per head accumulating the softmax denominator
    in PSUM row 64.
  - projections in bf16 (weights + activations), outputs fp32.
  - emission is software-pipelined: k/v projections of later quads are
    interleaved into the (Activation-bound) attention stream so the PE never
    sits behind exp in the in-order queue.
"""

import sys
import numpy as np

sys.path.insert(0, "/opt/trn_rl_repo")

import concourse.bacc as bacc  # noqa: E402
import concourse.tile as tile  # noqa: E402
from concourse import mybir  # noqa: E402

import ml_dtypes  # noqa: E402

F32 = mybir.dt.float32
BF16 = mybir.dt.bfloat16
F8 = mybir.dt.float8e4
AF = mybir.ActivationFunctionType
PM = mybir.MatmulPerfMode
BF16NP = ml_dtypes.bfloat16
F8NP = ml_dtypes.float8_e4m3fn

NHEAD = 16
DH = 64
B = 4
TQ = 2048
TKV = 2048
D = 1024
T_CORE = TQ // 2  # query rows per core
N_CORES = 8


def emit(nc, tc, hd, T, S, NH, phases=('q', 'kv', 'attn', 'out')):
    """Emit the per-core kernel.  T query rows, S kv rows, NH heads."""
    Dm = NH * DH
    NJ = Dm // 128          # 128-row feature blocks (8)
    NG = NH // 4            # head quads (4)
    NSB = S // 128          # kv 128-blocks (16)
    NTC = T // 512          # q 512-chunks (2)
    NKC = S // 512          # kv 512-chunks (4)
    scale = 1.0 / float(np.sqrt(DH))

    do_q = 'q' in phases
    do_kv = 'kv' in phases
    do_attn = 'attn' in phases
    do_out = 'out' in phases

    with (
        tc.tile_pool(name="consts", bufs=1) as consts,
        tc.tile_pool(name="pdata", bufs=1) as pdata,
        tc.tile_pool(name="wkvp", bufs=2) as wkvp,
        tc.tile_pool(name="ropep", bufs=3) as ropep,
        tc.tile_pool(name="e8p", bufs=3) as e8p,
        tc.tile_pool(name="nrmp", bufs=2) as nrmp,
        tc.tile_pool(name="ostg", bufs=2) as ostg,
        tc.tile_pool(name="psP", bufs=2, space="PSUM") as psP,
        tc.tile_pool(name="psS", bufs=2, space="PSUM") as psS,
        tc.tile_pool(name="psV", bufs=2, space="PSUM") as psV,
    ):
        # ---------------- constant loads ----------------
        bq_sb = consts.tile([128, NJ], F32, tag="bq")
        nc.sync.dma_start(bq_sb[:], hd["bq_t"][:])
        bk_sb = consts.tile([128, NJ], F32, tag="bk")
        nc.sync.dma_start(bk_sb[:], hd["bk_t"][:])
        bo_sb = consts.tile([128, NJ], F32, tag="bo")
        nc.sync.dma_start(bo_sb[:], hd["bo_t"][:])
        bv_sb = consts.tile([128, Dm], F32, tag="bv")
        nc.sync.dma_start(bv_sb[:], hd["bv_bcast"][:])
        ones_sb = consts.tile([128, NSB * NH], BF16, tag="ones")
        nc.sync.dma_start(ones_sb[:], hd["ones8"][:])
        crepq = consts.tile([128, 2 * T], BF16, tag="crepq")
        nc.scalar.dma_start(crepq[:], hd["crepq"][:])
        ssinq = consts.tile([128, 2 * T], BF16, tag="ssinq")
        nc.scalar.dma_start(ssinq[:], hd["ssinq"][:])
        crepk = consts.tile([128, 2 * S], BF16, tag="crepk")
        nc.scalar.dma_start(crepk[:], hd["crepk"][:])
        ssink = consts.tile([128, 2 * S], BF16, tag="ssink")
        nc.scalar.dma_start(ssink[:], hd["ssink"][:])

        # ---------------- persistent data ----------------
        ctx_sb = pdata.tile([128, NJ * S], BF16, tag="ctx")
        qT8 = pdata.tile([128, 2 * NG * T], F8, tag="qT8")     # (g,t) x T
        kT8 = pdata.tile([128, 2 * NG * S], F8, tag="kT8")     # (g,t) x S
        vq = pdata.tile([128, NSB * NH * 65], BF16, tag="vq")    # (sb,h,65)

        # ones columns of vq (softmax denominator trick)
        nc.vector.tensor_copy(
            vq[:].rearrange("p (s h c) -> p s h c", h=NH, c=65)[:, :, :, 64:65],
            ones_sb[:].rearrange("p (s h) -> p s h", h=NH)[:, :, :, None])

        # ---------------- task bodies ----------------
        def rope_project(dst8, wslice, b_sb, src_sb, crep, ssin, g, c, L):
            """Project block (g, t=0/1) for 512-col chunk c and apply RoPE."""
            raws = ropep.tile([128, 1024], BF16, tag="raw")
            for t in range(2):
                ps = psP.tile([128, 512], F32, tag="pp")
                col = g * 2 + t
                for i in range(NJ):
                    nc.tensor.matmul(
                        ps[:], wslice(i, t),
                        src_sb[:, i * L + c * 512:i * L + c * 512 + 512],
                        start=(i == 0), stop=(i == NJ - 1))
                nc.vector.tensor_scalar_add(
                    raws[:, t * 512:t * 512 + 512], ps[:],
                    b_sb[:, col:col + 1])
            for t in range(2):
                pa = ropep.tile([128, 512], BF16, tag="pa")
                pb = ropep.tile([128, 512], BF16, tag="pb")
                nc.vector.tensor_mul(
                    pa[:], raws[:, t * 512:t * 512 + 512],
                    crep[:, t * L + c * 512:t * L + c * 512 + 512])
                nc.vector.tensor_mul(
                    pb[:], raws[:, (t ^ 1) * 512:(t ^ 1) * 512 + 512],
                    ssin[:, t * L + c * 512:t * L + c * 512 + 512])
                nc.vector.tensor_add(
                    dst8[:, (g * 2 + t) * L + c * 512:
                         (g * 2 + t) * L + c * 512 + 512],
                    pa[:], pb[:])

        wk_cur = {}
        wv_cur = {}

        def load_wk(g):
            wk = wkvp.tile([128, NJ * 256], BF16, tag="wk")
            nc.sync.dma_start(
                wk[:].rearrange("p (a c) -> p a c", a=NJ),
                hd["wk"][:, g * 256:(g + 1) * 256].rearrange(
                    "(a p) c -> p a c", p=128))
            wk_cur[g] = wk

        def load_wv(gp):
            wv = wkvp.tile([128, NJ * 512], BF16, tag="wv")
            nc.sync.dma_start(
                wv[:].rearrange("p (a c) -> p a c", a=NJ),
                hd["wv"][:, gp * 512:(gp + 1) * 512].rearrange(
                    "(a p) c -> p a c", p=128))
            wv_cur[gp] = wv

        def k_task(g, c):
            wk = wk_cur[g]
            rope_project(
                kT8,
                lambda i, t: wk[:, i * 256 + t * 128:i * 256 + t * 128 + 128],
                bk_sb, ctx_sb, crepk, ssink, g, c, S)

        def v_task(gp, sb):
            """v projection for head-octet gp (heads gp*8..), kv block sb."""
            ps = psP.tile([128, 512], F32, tag="pp")
            wv = wv_cur[gp]
            for i in range(NJ):
                nc.tensor.matmul(
                    ps[:],
                    ctx_sb[:, i * S + sb * 128:i * S + sb * 128 + 128],
                    wv[:, i * 512:(i + 1) * 512],
                    start=(i == 0), stop=(i == NJ - 1))
            dst = vq[:, sb * (NH * 65) + gp * 520:
                     sb * (NH * 65) + gp * 520 + 520].rearrange(
                "p (h c) -> p h c", c=65)[:, :, 0:64]
            nc.vector.tensor_add(
                dst, ps[:].rearrange("p (h c) -> p h c", c=64),
                bv_sb[:, gp * 512:(gp + 1) * 512].rearrange(
                    "p (h c) -> p h c", c=64))

        pv_cur = {}

        def attn_task(h, sbp, qT8_, a_sb_):
            """scores + exp + PV for head h, kv block pair sbp."""
            g, s = h // 4, h % 4
            rs = slice(s * 32, s * 32 + 32)
            if sbp == 0:
                pv_cur[h] = [psV.tile([65, 512], F32, tag="pv", name="pv")
                             for _ in range(NTC)]
            for k in range(2):
                sb = sbp * 2 + k
                ps = psS.tile([128, 1024], F32, tag="sc")
                lhs = kT8[rs, :].rearrange("p (u v) -> p u v", v=S)[
                    :, g * 2:g * 2 + 2, sb * 128:sb * 128 + 128]
                for c in range(NTC):
                    rhs = qT8_[rs, :].rearrange("p (u v) -> p u v", v=T)[
                        :, g * 2:g * 2 + 2, c * 512:c * 512 + 512]
                    nc.tensor.matmul(
                        ps[:, c * 512:c * 512 + 512], lhs, rhs,
                        start=True, stop=True, perf_mode=PM.DoubleRow,
                        tile_position=(s * 32, 0))
                e8 = e8p.tile([128, 1024], BF16, tag="e8", name="e8")
                nc.scalar.activation(e8[:], ps[:], AF.Exp, scale=scale)
                lhsv = vq[:].rearrange(
                    "p (sb h c) -> p sb h c", h=NH, c=65)[:, sb, h, :]
                for c in range(NTC):
                    nc.tensor.matmul(
                        pv_cur[h][c][:], lhsv,
                        e8[:, c * 512:c * 512 + 512],
                        start=(sb == 0), stop=(sb == NSB - 1))

        def norm_task(h, a_sb_):
            """normalize PV by the denominator; place into a_sb."""
            b, hp = h // 2, h % 2
            for c in range(NTC):
                ps = pv_cur[h][c]
                zinv = nrmp.tile([1, 512], F32, tag="zinv")
                nc.vector.reciprocal(zinv[:], ps[64:65, :])
                bc = nrmp.tile([64, 512], F32, tag="bc")
                nc.gpsimd.partition_broadcast(bc[:], zinv[:])
                dst_cols = slice(b * T + c * 512, b * T + c * 512 + 512)
                if hp == 0:
                    nc.vector.tensor_mul(
                        a_sb_[0:64, dst_cols], ps[0:64, :], bc[:])
                else:
                    st = nrmp.tile([64, 512], BF16, tag="st")
                    nc.vector.tensor_mul(st[:], ps[0:64, :], bc[:])
                    nc.sync.dma_start(a_sb_[64:128, dst_cols], st[:])

        def out_task(e, c, wo_sb_, a_sb_):
            ps = psP.tile([128, 512], F32, tag="pp")
            for i in range(NJ):
                nc.tensor.matmul(
                    ps[:],
                    wo_sb_[:, i * Dm + e * 128:i * Dm + e * 128 + 128],
                    a_sb_[:, i * T + c * 512:i * T + c * 512 + 512],
                    start=(i == 0), stop=(i == NJ - 1))
            ot = ostg.tile([128, 512], F32, tag="ot")
            nc.vector.tensor_scalar_add(ot[:], ps[:], bo_sb[:, e:e + 1])
            nc.sync.dma_start(
                hd["out_t"][e * 128:(e + 1) * 128, c * 512:c * 512 + 512],
                ot[:])

        def emit_sched(attn_tasks, dl_tasks):
            """Emit attn_tasks in order; dl_tasks is [(deadline_ai|None, fn)].
            A task with deadline d is emitted before attn_tasks[d]; None
            tasks are spread evenly over the stream."""
            nones = [f for d, f in dl_tasks if d is None]
            per = {}
            for d, f in dl_tasks:
                if d is not None:
                    per.setdefault(d, []).append(f)
            na = len(attn_tasks)
            k = 0
            for ai, t in enumerate(attn_tasks):
                for f in per.pop(ai, []):
                    f()
                t()
                want = ((ai + 1) * len(nones)) // max(na, 1)
                while k < want:
                    nones[k]()
                    k += 1
            for d in sorted(per):
                for f in per[d]:
                    f()
            while k < len(nones):
                nones[k]()
                k += 1

        # ---------------- phase 1: q projection (x, wq scoped) ----------
        with tc.tile_pool(name="ph1", bufs=1) as ph1:
            x_sb = ph1.tile([128, NJ * T], BF16, tag="x")
            for a in range(NJ):
                nc.gpsimd.dma_start(
                    x_sb[:, a * T:(a + 1) * T],
                    hd["xT"][a * 128:(a + 1) * 128, :])
            for a in range(NJ):
                nc.gpsimd.dma_start(
                    ctx_sb[:, a * S:(a + 1) * S],
                    hd["ctxT"][a * 128:(a + 1) * 128, :])
            wq_sb = ph1.tile([128, NJ * Dm], BF16, tag="wq")
            for a in range(NJ):
                nc.sync.dma_start(
                    wq_sb[:, a * Dm:(a + 1) * Dm],
                    hd["wq"][a * 128:(a + 1) * 128, :])
            if do_q:
                for g in range(NG):
                    for c in range(NTC):
                        rope_project(
                            qT8,
                            (lambda i, t, gg=g:
                             wq_sb[:, i * Dm + (gg * 2 + t) * 128:
                                   i * Dm + (gg * 2 + t) * 128 + 128]),
                            bq_sb, x_sb, crepq, ssinq, g, c, T)

        # ---------------- late pool reuses phase-1 space ----------------
        with tc.tile_pool(name="late", bufs=1) as late:
            a_sb = late.tile([128, NJ * T], BF16, tag="a")
            wo_sb = late.tile([128, NJ * Dm], BF16, tag="wo")
            nc.scalar.dma_start(
                wo_sb[:].rearrange("p (a c) -> p a c", a=NJ),
                hd["wo"][:].rearrange("(a p) c -> p a c", p=128))

            if do_kv:
                load_wk(0)
                load_wv(0)
                k_task(0, 0)
                v_task(0, 0)
                v_task(0, 1)

            if do_attn:
                for g in range(NG):
                    attn_tasks = []
                    for h in range(g * 4, g * 4 + 4):
                        for sbp in range(NSB // 2):
                            attn_tasks.append(
                                (lambda hh=h, ss=sbp:
                                 attn_task(hh, ss, qT8, a_sb)))
                        attn_tasks.append(
                            (lambda hh=h: norm_task(hh, a_sb)))
                    dl = []
                    if do_kv:
                        if g == 0:
                            # head 0 (ai = sbp for sbp 0..7) touches every
                            # kv block: meet its per-sbp deadlines.
                            dl += [(max(2 * c - 1, 0),
                                    lambda c=c: k_task(0, c))
                                   for c in range(1, NKC)]
                            dl += [(max(sb // 2 - 1, 0),
                                    lambda sb=sb: v_task(0, sb))
                                   for sb in range(2, NSB)]
                            dl.append((None, lambda: load_wk(1)))
                            dl += [(None, lambda c=c: k_task(1, c))
                                   for c in range(NKC)]
                            dl.append((None, lambda: load_wv(1)))
                        elif g == 1:
                            dl += [(None, lambda sb=sb: v_task(1, sb))
                                   for sb in range(NSB)]
                            dl.append((None, lambda: load_wk(2)))
                            dl += [(None, lambda c=c: k_task(2, c))
                                   for c in range(NKC)]
                        elif g == 2:
                            dl.append((None, lambda: load_wk(3)))
                            dl += [(None, lambda c=c: k_task(3, c))
                                   for c in range(NKC)]
                    emit_sched(attn_tasks, dl)
            elif do_kv:
                for c in range(1, NKC):
                    k_task(0, c)
                for sb in range(2, NSB):
                    v_task(0, sb)
                load_wv(1)
                for sb in range(NSB):
                    v_task(1, sb)
                for g in range(1, NG):
                    load_wk(g)
                    for c in range(NKC):
                        k_task(g, c)

            if do_out:
                for e in range(NJ):
                    for c in range(NTC):
                        out_task(e, c, wo_sb, a_sb)


def build(T=T_CORE, S=TKV, NH=NHEAD, reps=1,
          phases=('q', 'kv', 'attn', 'out')):
    Dm = NH * DH
    nc = bacc.Bacc("TRN2", target_bir_lowering=False, debug=False)
    hd = {}
    for name, shape, dt in [
        ("xT", [Dm, T], BF16), ("ctxT", [Dm, S], BF16),
        ("wq", [Dm, Dm], BF16), ("wk", [Dm, Dm], BF16),
        ("wv", [Dm, Dm], BF16), ("wo", [Dm, Dm], BF16),
        ("crepk", [128, 2 * S], BF16), ("ssink", [128, 2 * S], BF16),
        ("crepq", [128, 2 * T], BF16), ("ssinq", [128, 2 * T], BF16),
        ("bq_t", [128, Dm // 128], F32), ("bk_t", [128, Dm // 128], F32),
        ("bv_bcast", [128, Dm], F32), ("bo_t", [128, Dm // 128], F32),
        ("ones8", [128, (S // 128) * NH], BF16),
    ]:
        hd[name] = nc.dram_tensor(name, shape, dt, kind="ExternalInput")
    hd["out_t"] = nc.dram_tensor("out_t", [Dm, T], F32, kind="ExternalOutput")

    with tile.TileContext(nc) as tc:
        for _ in range(reps):
            emit(nc, tc, hd, T, S, NH, phases=phases)
    nc.compile()
    return nc


def host_prep(x, context, Wq, bq, Wkv, bkv, Wo, bo, cos_tab, sin_tab,
              T=T_CORE, S=TKV, NH=NHEAD, n_cores=N_CORES):
    """Build the per-core input maps (layout + dtype work, no math)."""
    Dm = NH * DH
    # DR feature order: block (g, t), partition row s*32 + j
    #   <- original feature (g*4+s)*64 + 2*j + t
    g_i = np.arange(Dm) // 256
    t_i = (np.arange(Dm) // 128) % 2
    s_i = (np.arange(Dm) % 128) // 32
    j_i = np.arange(Dm) % 32
    perm = (g_i * 4 + s_i) * 64 + 2 * j_i + t_i
    c = np.ascontiguousarray

    def b16(a):
        return c(a.astype(BF16NP))

    wq = b16(Wq[perm, :].T)
    wk = b16(Wkv[0:Dm][perm, :].T)
    wv = b16(Wkv[Dm:2 * Dm].T)
    wo = b16(Wo.T)
    bq_t = c(bq[perm].reshape(Dm // 128, 128).T.astype(np.float32))
    bk_t = c(bkv[0:Dm][perm].reshape(Dm // 128, 128).T.astype(np.float32))
    bv_bcast = c(np.tile(bkv[Dm:2 * Dm].reshape(1, Dm),
                         (128, 1)).astype(np.float32))
    bo_t = c(bo.reshape(Dm // 128, 128).T.astype(np.float32))

    def mk_tables(lo, hi):
        # [128, 2*L]: rows s*32+j -> freq j; col t*L+pos
        ct = np.tile(cos_tab[lo:hi].T, (4, 1)).astype(np.float32)  # (128, L)
        st = np.tile(sin_tab[lo:hi].T, (4, 1)).astype(np.float32)
        crep = np.concatenate([ct, ct], axis=1)
        ssin = np.concatenate([-st, st], axis=1)
        return b16(crep), b16(ssin)

    crepk, ssink = mk_tables(0, S)
    ones8 = np.ones((128, (S // 128) * NH), dtype=BF16NP)

    shared = dict(wq=wq, wk=wk, wv=wv, wo=wo, bq_t=bq_t, bk_t=bk_t,
                  bv_bcast=bv_bcast, bo_t=bo_t, crepk=crepk, ssink=ssink,
                  ones8=ones8)
    in_maps = []
    halves = n_cores // x.shape[0]
    for core in range(n_cores):
        b_i, th = divmod(core, halves)
        crepq, ssinq = mk_tables(th * T, (th + 1) * T)
        m = dict(shared)
        m.update(
            xT=b16(x[b_i, th * T:(th + 1) * T, :].T),
            ctxT=b16(context[b_i].T),
            crepq=crepq, ssinq=ssinq,
        )
        in_maps.append(m)
    return in_maps


_NC_CACHE = {}


def get_nc():
    if "nc" not in _NC_CACHE:
        _NC_CACHE["nc"] = build()
    return _NC_CACHE["nc"]


def make_runner(nc, n_cores=N_CORES):
    """Build a reusable jitted SPMD executor (device-resident inputs)."""
    import jax
    from jax.experimental.shard_map import shard_map
    from jax.sharding import Mesh, NamedSharding, PartitionSpec
    from concourse import bass2jax, mybir as _mybir

    bass2jax.install_neuronx_cc_hook()
    part_name = (nc.partition_id_tensor.name
                 if nc.partition_id_tensor else None)
    in_names, out_names, out_avals = [], [], []
    for alloc in nc.m.functions[0].allocations:
        if not isinstance(alloc, _mybir.MemoryLocationSet):
            continue
        name = alloc.memorylocations[0].name
        if alloc.kind == "ExternalInput":
            if name == part_name:
                continue
            in_names.append(name)
        elif alloc.kind == "ExternalOutput":
            out_names.append(name)
            out_avals.append(jax.core.ShapedArray(
                tuple(alloc.tensor_shape), _mybir.dt.np(alloc.dtype)))
    n_params = len(in_names)
    all_in = in_names + out_names
    if part_name is not None:
        all_in = all_in + [part_name]

    def _body(*args):
        ops = list(args)
        if part_name is not None:
            ops.append(bass2jax.partition_id_tensor())
        outs = bass2jax._bass_exec_p.bind(
            *ops,
            out_avals=tuple(out_avals),
            in_names=tuple(all_in),
            out_names=tuple(out_names),
            lowering_input_output_aliases=(),
            sim_require_finite=True,
            sim_require_nnan=True,
            nc=nc,
        )
        return tuple(outs)

    devices = jax.devices()[:n_cores]
    mesh = Mesh(np.asarray(devices), ("core",))
    nouts = len(out_names)
    sharded = jax.jit(
        shard_map(_body, mesh=mesh,
                  in_specs=(PartitionSpec("core"),) * (n_params + nouts),
                  out_specs=(PartitionSpec("core"),) * nouts,
                  check_rep=False),
        keep_unused=True,
    )
    sh = NamedSharding(mesh, PartitionSpec("core"))

    def put(in_maps):
        args = [np.concatenate([m[name] for m in in_maps], axis=0)
                for name in in_names[:n_params]]
        for av in out_avals:
            args.append(np.zeros((n_cores * av.shape[0],) + av.shape[1:],
                                 av.dtype))
        return [jax.device_put(a, sh) for a in args]

    def run(args):
        outs = sharded(*args)
        jax.block_until_ready(outs)
        return outs

    def gather(outs):
        return [
            {name: np.asarray(outs[i]).reshape(n_cores, *out_avals[i].shape)[c]
             for i, name in enumerate(out_names)}
            for c in range(n_cores)
        ]

    return put, run, gather


def get_runner():
    if "runner" not in _NC_CACHE:
        _NC_CACHE["runner"] = make_runner(get_nc())
    return _NC_CACHE["runner"]


def kernel(x, context, Wq, bq, Wkv, bkv, Wo, bo, cos_tab, sin_tab):
    args = [np.asarray(a, dtype=np.float32) for a in
            (x, context, Wq, bq, Wkv, bkv, Wo, bo, cos_tab, sin_tab)]
    in_maps = host_prep(*args)
    put, run, gather = get_runner()
    res = gather(run(put(in_maps)))
    out = np.empty((B, TQ, D), dtype=np.float32)
    for core in range(N_CORES):
        b_i, th = divmod(core, 2)
        out[b_i, th * T_CORE:(th + 1) * T_CORE, :] = res[core]["out_t"].T
    return out
